# revision 2
# baseline (speedup 1.0000x reference)
"""MultiHeadSelectiveAttention TRN2 kernel v2: fp16 single-pass pipeline.

Shards batch (B=8) across 8 NeuronCores (one element per core). Math per
batch b (value-head-dim-1 collapse, identical algebra to the reference):
    v   = x Wv + bv                       [L, H]
    xv  = x^T v                           [D, H]
    ktv = blockdiag_mask(Wk^T xv + bk (x) sum_l v)   [D, H]
    U   = Wq ktv ; c[h] = bq . ktv[:, h]
    out = sigmoid((x U + c)/8)^T * mask   [H, L]
All HBM traffic and PE streams are fp16 (x, Wk, Wq^T pre-cast on host);
PSUM accumulation is fp32. Wq arrives pre-transposed so no on-chip
weight transpose is needed. Weight tiles are prefetched during phase A.
"""
import sys
sys.path.insert(0, '/opt/trn_rl_repo')
from contextlib import ExitStack
import numpy as np
import concourse.bass as bass
import concourse.tile as tile
import concourse.mybir as mybir
from concourse.tile import ScopedClock
from concourse.masks import make_identity

f32 = mybir.dt.float32
f16 = mybir.dt.float16
Sigmoid = mybir.ActivationFunctionType.Sigmoid
Copy = mybir.ActivationFunctionType.Copy

B = 8
L, D, H = 4096, 1024, 16
NLT, NDT = L // 128, D // 128   # 32, 8
BLK = 4                          # l-tiles per block
NBLK = NLT // BLK                # 8

_wait_fix_counter = [0]
SPLIT_WAITS = [True]

def _split_multi_waits(nc):
    for f in nc.m.functions:
        for bb in f.blocks:
            new_insts = []
            for inst in bb.instructions:
                si = getattr(inst, 'sync_info', None)
                if si is not None and len(si.on_wait) > 1:
                    waits = list(si.on_wait)
                    for w in waits[:-1]:
                        _wait_fix_counter[0] += 1
                        nop = mybir.InstNoOp(
                            name=f"waitfix-{_wait_fix_counter[0]}",
                            engine=inst.engine, opcode="NoOp", ins=[], outs=[],
                            sync_info=mybir.SyncInfo(on_wait=[w], on_update=[]),
                        )
                        new_insts.append(nop)
                    inst.sync_info = mybir.SyncInfo(
                        on_wait=[waits[-1]], on_update=list(si.on_update))
                new_insts.append(inst)
            bb.instructions[:] = new_insts

def _drain_and_barrier_split(self, tick_clock, wait_clock):
    nc = self.nc
    probe = nc.sync.nop()
    wait_clock.add_sem_waits(probe.ins, ScopedClock({None: tick_clock.global_clock}))
    nc.sync.drain()
    nc.all_engine_barrier()
    assert self.sems is not None
    popped = nc._tile_sem_poison_stack.pop()
    assert popped is self._sem_poison
    nc.clear_and_free_semaphores(list(self.sems.allocated().values()))
    nc.all_engine_barrier()
    if SPLIT_WAITS[0]:
        _split_multi_waits(nc)

tile.TileContext._drain_and_barrier = _drain_and_barrier_split


def build(dump=()):
    nc = bass.Bass(trn_type="TRN2")
    x = nc.dram_tensor("x", [L, D], f16, kind="ExternalInput")
    wk = nc.dram_tensor("wk", [D, D], f16, kind="ExternalInput")      # Wk natural
    wqt = nc.dram_tensor("wqt", [D, D], f16, kind="ExternalInput")    # Wq^T
    # packed fp16 constants: [bvt (16) | bqp (8) | wvp (128)]
    k16 = nc.dram_tensor("k16", [128, H + NDT + NDT * H], f16, kind="ExternalInput")
    kf = nc.dram_tensor("kf", [H, 1], f32, kind="ExternalInput")  # bv column
    bkr = nc.dram_tensor("bkr", [1, D], f16, kind="ExternalInput")
    out = nc.dram_tensor("out", [H, L], f16, kind="ExternalOutput")
    dumps = {}
    for name, shape, dt_ in [("v", [L, H], f32), ("xvt", [H, D], f32),
                             ("ktvbdt", [H, D], f32), ("ut", [H, D], f32),
                             ("c", [H, 1], f32)]:
        if name in dump or (name == "c" and "ut" in dump):
            dumps[name] = nc.dram_tensor("d_" + name, shape, dt_, kind="ExternalOutput")

    with ExitStack() as ctx:
        tc = ctx.enter_context(tile.TileContext(nc))
        konst = ctx.enter_context(tc.tile_pool(name="konst", bufs=1))
        xtrp = ctx.enter_context(tc.tile_pool(name="xtr", bufs=1))
        wpool = ctx.enter_context(tc.tile_pool(name="wpool", bufs=1))
        pers = ctx.enter_context(tc.tile_pool(name="pers", bufs=1))
        ps_xv = ctx.enter_context(tc.tile_pool(name="ps_xv", bufs=1, space="PSUM"))

        # ---------------- constants (no DMA yet) ----------------
        ident = konst.tile([128, 128], f32)
        make_identity(nc, ident[:])
        ident16 = konst.tile([128, 128], f16)
        nc.vector.tensor_copy(ident16[:], ident[:])
        k16_sb = konst.tile([128, H + NDT + NDT * H], f16)
        bvt_sb = k16_sb[:, 0:H]
        bq_sb = k16_sb[:, H:H + NDT]
        wv_sb = k16_sb[:, H + NDT:H + NDT + NDT * H]
        kf_sb = konst.tile([H, 1], f32)
        bvcol = kf_sb[:, 0:1]
        bkr_sb = konst.tile([1, D], f16)
        # block-diag selection mask, packed [128, 8*16]: block k keeps
        # rows 0:64 -> col 2k, rows 64:128 -> col 2k+1
        bdm = konst.tile([128, NDT * H], f32)
        nc.vector.memset(bdm[:], 0.0)
        for k in range(NDT):
            nc.vector.memset(bdm[0:64, 16 * k + 2 * k:16 * k + 2 * k + 1], 1.0)
            nc.vector.memset(bdm[64:128, 16 * k + 2 * k + 1:16 * k + 2 * k + 2], 1.0)

        # persistent x^T tiles and weight tiles
        xtr = [xtrp.tile([128, L], f16, name=f"xtr{d}", tag=f"xtr{d}") for d in range(NDT)]
        wk_t = [wpool.tile([128, D], f16, name=f"wkt{k}", tag=f"wkt{k}") for k in range(NDT)]
        wqt_t = [wpool.tile([128, D], f16, name=f"wqtt{k}", tag=f"wqtt{k}") for k in range(NDT)]
        xvt_ps = [ps_xv.tile([H, 512], f32, name=f"xv{c}", tag=f"xv{c}") for c in range(2)]
        svacc = pers.tile([H, 1], f32, tag="svacc")
        n_xv = [0]

        # ---------------- PHASE A ----------------
        with tc.tile_pool(name="phA", bufs=2) as sbA, \
             tc.tile_pool(name="xnatp", bufs=3) as xnatp, \
             tc.tile_pool(name="vpbp", bufs=4) as vpbp, \
             tc.tile_pool(name="ps_tr", bufs=3, space="PSUM") as ps_tr, \
             tc.tile_pool(name="ps_v", bufs=2, space="PSUM") as ps_v, \
             tc.tile_pool(name="ps_f", bufs=1, space="PSUM") as ps_f:
            for blk in range(NBLK):
                xblk = xnatp.tile([128, BLK * D], f16, tag="xnat")
                if blk == 0:
                    # split the first load so transposes can start sooner
                    for half in range(2):
                        nc.sync.dma_start(
                            xblk[:, 2 * D * half:2 * D * half + 2 * D]
                            .rearrange("p (j d) -> p j d", j=2),
                            x[256 * half:256 * half + 256, :]
                            .rearrange("(j p) d -> p j d", p=128))
                else:
                    nc.sync.dma_start(
                        xblk[:].rearrange("p (j d) -> p j d", j=BLK),
                        x[512 * blk:512 * blk + 512, :]
                        .rearrange("(j p) d -> p j d", p=128))
                # constants trickle in on the idle Pool queue, x first
                if blk == 0:
                    nc.gpsimd.dma_start(k16_sb[:], k16[:, :])
                elif blk == 1:
                    nc.gpsimd.dma_start(kf_sb[:], kf[:, :])
                elif blk == 2:
                    nc.gpsimd.dma_start(bkr_sb[:], bkr[:, :])
                # prefetch weights under phase A compute, one block behind
                # the x loads so they never delay an x block
                for wb in ([blk - 1] if blk >= 1 else []) + \
                          ([NBLK - 1] if blk == NBLK - 1 else []):
                    nc.sync.dma_start(wk_t[wb][:], wk[128 * wb:128 * wb + 128, :])
                    nc.sync.dma_start(wqt_t[wb][:], wqt[128 * wb:128 * wb + 128, :])
                xnat = [xblk[:, D * j:D * (j + 1)] for j in range(BLK)]
                for d in range(NDT):
                    ps = ps_tr.tile([128, 512], f16, tag="tr")
                    for j in range(BLK):
                        nc.tensor.matmul(
                            ps[:, 128 * j:128 * j + 128],
                            xnat[j][:, 128 * d:128 * d + 128],
                            ident16[:],
                            start=True, stop=True, is_transpose=True,
                            skip_group_check=True)
                    if d % 8 in (0, 3, 6):
                        nc.scalar.copy(xtr[d][:, 512 * blk:512 * blk + 512], ps[:])
                    else:
                        nc.vector.tensor_copy(xtr[d][:, 512 * blk:512 * blk + 512], ps[:])
                # P1 / v-fold / P2, in one sub-chunk for blocks 0..6 and two
                # half-chunks for the last block (shorter exposed tail)
                subs = [(0, BLK)] if blk < NBLK - 1 else [(0, 2), (2, 2)]
                for j0, nj in subs:
                    ncols = 128 * nj
                    col0 = 512 * blk + 128 * j0
                    psv = ps_v.tile([H, ncols], f32, tag="v")
                    for d in range(NDT):
                        nc.tensor.matmul(
                            psv[:], wv_sb[:, H * d:H * d + H],
                            xtr[d][:, col0:col0 + ncols],
                            start=(d == 0), stop=(d == NDT - 1))
                    vts = sbA.tile([H, ncols], f16, tag="vts")
                    svp = sbA.tile([H, 1], f32, name="svp", tag="svp", bufs=2)
                    if j0 == 0:
                        nc.scalar.activation(vts[:], psv[:], Copy, accum_out=svp[:])
                    else:
                        # last half-chunk: keep the scalar engine free
                        nc.vector.tensor_copy(vts[:], psv[:])
                        nc.vector.tensor_reduce(
                            svp[:], psv[:], mybir.AxisListType.X,
                            mybir.AluOpType.add)
                    if blk == 0 and j0 == 0:
                        nc.vector.tensor_copy(svacc[:], svp[:])
                    else:
                        nc.vector.tensor_add(svacc[:], svacc[:], svp[:])
                    # fold-transpose to v-natural [128, 16] per l-tile, + bv
                    psf = ps_f.tile([128, nj * H], f16, tag="vf")
                    for j in range(nj):
                        nc.tensor.matmul(
                            psf[:, H * j:H * j + H],
                            vts[:, 128 * j:128 * j + 128],
                            ident16[0:H, 0:H],
                            start=True, stop=True, is_transpose=True,
                            skip_group_check=True)
                    vpb = vpbp.tile([128, nj * H], f16, tag="vpb")
                    nc.vector.tensor_add(
                        vpb[:].rearrange("p (j h) -> p j h", j=nj),
                        psf[:].rearrange("p (j h) -> p j h", j=nj),
                        bvt_sb[:].unsqueeze(1).broadcast_to([128, nj, H]))
                    if "v" in dump:
                        for j in range(nj):
                            lt = BLK * blk + j0 + j
                            nc.gpsimd.dma_start(
                                dumps["v"][128 * lt:128 * lt + 128, :],
                                vpb[:, H * j:H * j + H])
                    # P2: xv accumulation over l-tiles
                    for j in range(nj):
                        n_xv[0] += 1
                        for c in range(2):
                            nc.tensor.matmul(
                                xvt_ps[c][:], vpb[:, H * j:H * j + H],
                                xnat[j0 + j][:, 512 * c:512 * c + 512],
                                start=(n_xv[0] == 1), stop=(n_xv[0] == NLT))

        # ---------------- PHASE B ----------------
        xvt = pers.tile([H, D], f32, tag="xvt")
        nc.scalar.copy(xvt[:, 0:512], xvt_ps[0][:])
        nc.vector.tensor_copy(xvt[:, 512:1024], xvt_ps[1][:])
        sv = pers.tile([H, 1], f32, tag="sv")
        svb = pers.tile([H, 1], f32, tag="svb")
        nc.scalar.activation(svb[:], bvcol, Copy, scale=float(L))
        nc.vector.tensor_add(sv[:], svacc[:], svb[:])
        # sv as an fp16 row [1, 16] for the bk (x) sv outer product
        svr = pers.tile([1, H], f16, tag="svr")
        if "xvt" in dump:
            nc.gpsimd.dma_start(dumps["xvt"][:, :], xvt[:])

        with tc.tile_pool(name="phB", bufs=2) as sbB:
            # xv pair tiles: transpose xvt into one [128, 128] psum, cast fp16
            xvp_sb = sbB.tile([128, NDT * H], f16, tag="xvp", bufs=1)
            with tc.tile_pool(name="ps_m1", bufs=1, space="PSUM") as ps_m:
                psm = ps_m.tile([128, NDT * H + H], f32, tag="sm")
                for d in range(NDT):
                    nc.tensor.matmul(
                        psm[:, H * d:H * d + H],
                        xvt[0:H, 128 * d:128 * d + 128], ident[0:H, 0:H],
                        start=True, stop=True, is_transpose=True,
                        skip_group_check=True)
                nc.vector.tensor_copy(xvp_sb[:], psm[:, 0:NDT * H])
                # sv row: [16,1] -> [1,16] on the same psum bank
                nc.tensor.matmul(
                    psm[0:1, NDT * H:NDT * H + H], sv[:], ident[0:H, 0:H],
                    start=True, stop=True, is_transpose=True,
                    skip_group_check=True)
                nc.vector.tensor_copy(svr[:], psm[0:1, NDT * H:NDT * H + H])
            # step 3: KTVfullT = xv^T Wk + sv (x) bk
            ps_s3 = ctx.enter_context(tc.tile_pool(name="ps_s3", bufs=1, space="PSUM"))
            ps3 = [ps_s3.tile([H, 512], f32, name=f"s3{c}", tag=f"s3{c}") for c in range(2)]
            for k in range(NDT):
                for c in range(2):
                    nc.tensor.matmul(
                        ps3[c][:], xvp_sb[:, H * k:H * k + H],
                        wk_t[k][:, 512 * c:512 * c + 512],
                        start=(k == 0), stop=False)
            for c in range(2):
                nc.tensor.matmul(
                    ps3[c][:], svr[:], bkr_sb[:, 512 * c:512 * c + 512],
                    start=False, stop=True)
            ktvbdt = sbB.tile([H, D], f32, tag="ktvbdt", bufs=1)
            nc.scalar.copy(ktvbdt[:, 0:512], ps3[0][:])
            nc.vector.tensor_copy(ktvbdt[:, 512:1024], ps3[1][:])
            if "ktvbdt" in dump:
                nc.gpsimd.dma_start(dumps["ktvbdt"][:, :], ktvbdt[:])
            # ktv pair tiles (block-diag masked, fp16)
            ktvp_sb = sbB.tile([128, NDT * H], f16, tag="ktvp", bufs=1)
            with tc.tile_pool(name="ps_m2", bufs=1, space="PSUM") as ps_m:
                psm = ps_m.tile([128, NDT * H], f32, tag="sm")
                for k in range(NDT):
                    nc.tensor.matmul(
                        psm[:, H * k:H * k + H],
                        ktvbdt[0:H, 128 * k:128 * k + 128], ident[0:H, 0:H],
                        start=True, stop=True, is_transpose=True,
                        skip_group_check=True)
                nc.vector.tensor_mul(ktvp_sb[:], psm[:], bdm[:])
            # c = bq . ktvbd (then /8)
            cdiv8 = sbB.tile([H, 1], f32, tag="cdiv8", bufs=1)
            with tc.tile_pool(name="ps_c", bufs=1, space="PSUM") as ps_c:
                psc = ps_c.tile([H, 1], f32, tag="c", bufs=1)
                for k in range(NDT):
                    nc.tensor.matmul(
                        psc[:], ktvp_sb[:, H * k:H * k + H], bq_sb[:, k:k + 1],
                        start=(k == 0), stop=(k == NDT - 1))
                nc.scalar.activation(cdiv8[:], psc[:], Copy, scale=0.125)
            # step 4: UT = ktvbd^T Wq^T
            ps_s4 = ctx.enter_context(tc.tile_pool(name="ps_s4", bufs=1, space="PSUM"))
            ps4 = [ps_s4.tile([H, 512], f32, name=f"s4{c}", tag=f"s4{c}") for c in range(2)]
            for b in range(NDT):
                for c in range(2):
                    nc.tensor.matmul(
                        ps4[c][:], ktvp_sb[:, H * b:H * b + H],
                        wqt_t[b][:, 512 * c:512 * c + 512],
                        start=(b == 0), stop=(b == NDT - 1))
            ut = sbB.tile([H, D], f32, tag="ut", bufs=1)
            nc.scalar.copy(ut[:, 0:512], ps4[0][:])
            nc.vector.tensor_copy(ut[:, 512:1024], ps4[1][:])
            if "ut" in dump:
                nc.gpsimd.dma_start(dumps["ut"][:, :], ut[:])
                nc.gpsimd.dma_start(dumps["c"][:, :], cdiv8[:])
            # U pair tiles fp16
            upr_sb = sbB.tile([128, NDT * H], f16, tag="upr", bufs=1)
            with tc.tile_pool(name="ps_m3", bufs=1, space="PSUM") as ps_m:
                psm = ps_m.tile([128, NDT * H], f32, tag="sm")
                for d in range(NDT):
                    nc.tensor.matmul(
                        psm[:, H * d:H * d + H],
                        ut[0:H, 128 * d:128 * d + 128], ident[0:H, 0:H],
                        start=True, stop=True, is_transpose=True,
                        skip_group_check=True)
                nc.vector.tensor_copy(upr_sb[:], psm[:])
            # P5: zT chunks + sigmoid + mask + store (last chunk split in two
            # halves to shorten the exposed tail)
            ps_5 = ctx.enter_context(tc.tile_pool(name="ps_5", bufs=2, space="PSUM"))
            chunks = [(512 * ch, 512) for ch in range(7)] + [(3584, 256), (3840, 256)]
            for ci, (col0, ncols) in enumerate(chunks):
                ps5 = ps_5.tile([H, ncols], f32, name="ps5", tag="s5")
                for d in range(NDT):
                    nc.tensor.matmul(
                        ps5[:], upr_sb[:, H * d:H * d + H],
                        xtr[d][:, col0:col0 + ncols],
                        start=(d == 0), stop=(d == NDT - 1))
                sg = sbB.tile([H, ncols], f16, name="sg", tag="sg", bufs=4)
                nc.scalar.activation(sg[:], ps5[:], Sigmoid, bias=cdiv8[:], scale=0.125)
                st_eng = (nc.gpsimd, nc.scalar, nc.sync)[ci % 3]
                st_eng.dma_start(out[:, col0:col0 + ncols], sg[:])
    return nc, dumps


def ref_numpy(x, wq, wk, wv, bq, bk, bv):
    """f64 reference of the decomposed math for per-stage validation."""
    x64 = x.astype(np.float64)
    v = x64 @ wv.astype(np.float64) + bv.astype(np.float64)
    xv = x64.T @ v
    ktvfull = wk.astype(np.float64).T @ xv
    sv = v.sum(axis=0)
    ktvfull = ktvfull + np.outer(bk.astype(np.float64), sv)
    bd = np.zeros((D, H))
    for h in range(H):
        bd[64 * h:64 * h + 64, h] = 1.0
    ktvbd = ktvfull * bd
    u = wq.astype(np.float64) @ ktvbd
    c = bq.astype(np.float64) @ ktvbd
    z = (x64 @ u + c) / 8.0
    p = 1.0 / (1.0 + np.exp(-z))
    return dict(v=v, xvt=xv.T, ktvbdt=ktvbd.T, ut=u.T, c=c / 8.0, out=p.T)


_cache = {}

def _get_nc():
    if "nc" not in _cache:
        _cache["nc"] = build()[0]
    return _cache["nc"]


def make_in_maps(inputs):
    """Build per-core input maps from FULL reference inputs (host prep)."""
    x = np.asarray(inputs["x"], dtype=np.float32)
    Wq = np.asarray(inputs["Wq"], dtype=np.float32)
    Wk = np.asarray(inputs["Wk"], dtype=np.float32)
    Wv = np.asarray(inputs["Wv"], dtype=np.float32)
    bq = np.asarray(inputs["bq"], dtype=np.float32)
    bk = np.asarray(inputs["bk"], dtype=np.float32)
    bv = np.asarray(inputs["bv"], dtype=np.float32)
    wk16 = np.ascontiguousarray(Wk.astype(np.float16))
    wqt16 = np.ascontiguousarray(Wq.T.astype(np.float16))
    wvp = Wv.reshape(NDT, 128, H).transpose(1, 0, 2).reshape(128, NDT * H)
    bqp = bq.reshape(NDT, 128).T
    bvt = np.broadcast_to(bv[None, :], (128, H))
    k16 = np.ascontiguousarray(
        np.concatenate([bvt, bqp, wvp], axis=1).astype(np.float16))
    kf = np.ascontiguousarray(bv.reshape(H, 1))
    bkr = np.ascontiguousarray(bk.reshape(1, D).astype(np.float16))
    in_maps = []
    for b in range(B):
        in_maps.append({
            "x": np.ascontiguousarray(x[b].astype(np.float16)),
            "wk": wk16, "wqt": wqt16, "k16": k16, "kf": kf, "bkr": bkr,
        })
    return in_maps


def kernel(x, mask, Wq, bq, Wk, bk, Wv, bv):
    from concourse.bass_utils import run_bass_kernel_spmd
    nc = _get_nc()
    in_maps = make_in_maps(dict(x=x, Wq=Wq, bq=bq, Wk=Wk,
                                bk=bk, Wv=Wv, bv=bv))
    res = run_bass_kernel_spmd(nc, in_maps, core_ids=list(range(B)))
    out = np.stack([res.results[b]["out"] for b in range(B)], axis=0)
    out = out.astype(np.float32)
    # masked_fill on host: p_attn = sigmoid(z) * mask (bool, [B, L])
    mask_f = np.asarray(mask).astype(np.float32)
    out *= mask_f[:, None, :]
    return out
